# revision 1
# baseline (speedup 1.0000x reference)
"""Trainium2 Bass kernel for causal self-attention with PoPE.

Reference computation (B=2, T=2048, C=1024, H=16, D=64):
  qkv = x @ w_attn.T ; split q,k,v ; heads
  mu_q = softplus(q); mu_k = softplus(k)
  q_real = mu_q * cos(t w); q_imag = mu_q * sin(t w)
  k_real = mu_k * cos(t w + d); k_imag = mu_k * sin(t w + d)   [d = clip(delta)]
  att = softmax_causal((q_real k_real + q_imag k_imag)/sqrt(D))
  y = att @ v ; out = y @ w_proj.T

Sharding: 8 cores = 2 batches x 4 head-groups (4 heads each). Each core
computes its batch's QKV for its heads, attention, and a partial c_proj
(its heads' input-channel rows of w_proj). Host sums the 4 partials per
batch.

Per-core dataflow (all matmuls float32r: full PE rate at free dim >=
256, ~FP22 mantissa):
  xT   [c, t]     x[b]^T, c on partitions (8 tiles of 128)
  qk_h [128, t]   rows 0:64 = q_h, 64:128 = k_h (d-major), psum
  mu_h = ln(exp(qk_h)+1)   (ACT, exp in-place on psum; same table set
                            as the attention exp -> no table switches)
  Qt_h [128, t]   rows 0:64 mu_q*cos(tw)/8, rows 64:128 mu_q*sin(tw)/8
  Kt_h [128, t]   rows 0:64 mu_k*cos(tw+d), rows 64:128 mu_k*sin(tw+d)
  S^T  [tk, tq]   = Kt^T @ Qt (single K=128 matmul per 128x512 block,
                   two tq blocks share one 2-bank psum tile)
  P = exp(S^T)    no max subtraction (scores bounded ~6); causal mask as
                  0/1 multiply on the 16 diagonal blocks only (gpsimd)
  y_aug^T [96,tq] += V_aug[tk]^T @ P : V_aug = [V | ones | zeros] so row
                  64 of the psum accumulates the softmax denominator
  normalize       recip(denom) -> PE outer-product broadcast -> multiply
  c_proj          out[t,e] psum += y_t[c,t]^T @ w_projT[c,e]

Attention loops j-pairs outer so only 2 y-psum banks are live, leaving
room to double-buffer the 2-bank S tiles (fewer, wider ACT exp ops).
"""

import math
import os
import sys

import numpy as np

for _p in ("/opt/trn_rl_repo",):
    if _p not in sys.path and os.path.isdir(_p):
        sys.path.insert(0, _p)

import concourse.tile as tile
from concourse import bacc
from concourse import mybir
from concourse import bass_utils

B, T, C = 2, 2048, 1024
H, D = 16, 64
BASE = 10000.0
N_CORES = 8
HPC = 4  # heads per core
NCT = 8  # c tiles (1024/128)
NTT = 16  # t tiles of 128

F32 = mybir.dt.float32
F32R = mybir.dt.float32r
AF = mybir.ActivationFunctionType


def build_module():
    nc = bacc.Bacc(
        "TRN2", target_bir_lowering=False, debug=False, num_devices=N_CORES
    )

    xT_d = nc.dram_tensor("xT", (NCT, 128, T), F32R, kind="ExternalInput").ap()
    wqk_d = nc.dram_tensor("wqk", (NCT, 128, 512), F32R, kind="ExternalInput").ap()
    wv_d = nc.dram_tensor("wv", (NCT, 128, 256), F32R, kind="ExternalInput").ap()
    w2t_d = nc.dram_tensor("w2t", (2, 128, 1024), F32R, kind="ExternalInput").ap()
    trig_d = nc.dram_tensor("trig", (128, T), F32, kind="ExternalInput").ap()
    ab_d = nc.dram_tensor("ab", (HPC, 128, T), F32, kind="ExternalInput").ap()
    cmask_d = nc.dram_tensor("cmask", (128, 128), F32, kind="ExternalInput").ap()
    out_d = nc.dram_tensor("out", (NTT, 128, 1024), F32, kind="ExternalOutput").ap()

    idm = list(range(32))

    with tile.TileContext(nc) as tc:
        with (
            tc.tile_pool(name="persist", bufs=1) as persist,
            tc.tile_pool(name="mupool", bufs=1) as mupool,
            tc.tile_pool(name="ps2", bufs=2, space="PSUM") as ps2,
            tc.tile_pool(name="ps1", bufs=4, space="PSUM") as ps1,
        ):
            # persistent tiles (live across phases)
            v_aug = persist.tile([128, NTT, HPC, 65], F32R)
            ones_t = persist.tile([128, 128], F32R)
            nc.vector.memset(ones_t.bitcast(F32), 1.0)
            # per head slab: cols 0:64 = V, col 64 = ones (the y matmul
            # then accumulates the softmax denominator in psum row 64)
            nc.vector.memset(
                v_aug.rearrange("p a b c -> p (a b) c")[:, :, 64:65].bitcast(F32),
                1.0,
            )

            trig = persist.tile([128, T], F32)
            nc.gpsimd.dma_start(trig, trig_d)

            mu = [mupool.tile([128, T], F32, name=f"mu{h}") for h in range(HPC)]

            # ---------------- Phase A: QKV projection ----------------
            with tc.tile_pool(name="phA", bufs=1) as pha:
                xT = pha.tile([128, NCT, T], F32R)
                wv = pha.tile([128, NCT, 256], F32R)
                wqk_pool = tc.tile_pool(name="wqkp", bufs=1)
                wqkp = wqk_pool.__enter__()
                wqk = wqkp.tile([128, NCT, 512], F32R)
                nc.scalar.dma_start(wqk, wqk_d.rearrange("o p e -> p o e"))
                engs = [nc.sync, nc.gpsimd, nc.scalar]
                for o in range(NCT):
                    engs[o % 3].dma_start(xT[:, o, :], xT_d[o])
                nc.sync.dma_start(wv, wv_d.rearrange("o p e -> p o e"))

                # q,k per head: 2-bank psum [128, 1024] per tb-pair.
                # softplus = ln(exp(x)+1): exp lands in mu, ln runs in
                # place afterwards, batched so the ACT table isn't
                # reloaded between alternating Exp/Ln ops.
                exp_handles = {}
                for h in range(HPC):
                    for tbp in range(2):
                        ps = ps2.tile([128, 1024], F32, tag="s2", name="ps_qk")
                        base = tbp * 1024
                        for c in range(NCT):
                            for half in range(2):
                                o0 = half * 512
                                nc.tensor.matmul(
                                    ps[:, o0 : o0 + 512],
                                    lhsT=wqk[:, c, h * 128 : (h + 1) * 128],
                                    rhs=xT[:, c, base + o0 : base + o0 + 512],
                                    start=(c == 0),
                                    stop=(c == NCT - 1),
                                )
                        exp_handles[(h, tbp)] = nc.scalar.activation(
                            mu[h][:, base : base + 1024], ps, AF.Exp
                        )
                from concourse.tile_rust import add_dep_helper

                for h in range(HPC):
                    for tbp in range(2):
                        base = tbp * 1024
                        ln = nc.scalar.activation(
                            mu[h][:, base : base + 1024],
                            mu[h][:, base : base + 1024],
                            AF.Ln,
                            bias=1.0,
                        )
                        # order Lns after the 2-head group's last Exp so the
                        # ACT table isn't reloaded between every Exp/Ln pair
                        grp_last = exp_handles[(h, 1)]
                        add_dep_helper(
                            ln.ins,
                            grp_last.ins,
                            sync=False,
                            reason="group softplus lns after exps (ACT tables)",
                        )

                wqk_pool.__exit__(None, None, None)

                # V in [t, e] layout -> v_aug columns 0:64 per head
                for tt in range(NTT):
                    psv = ps1.tile([128, 512], F32, tag="mm", name="ps_v")
                    for c in range(NCT):
                        nc.tensor.matmul(
                            psv[:, 0:256],
                            lhsT=xT[:, c, tt * 128 : (tt + 1) * 128],
                            rhs=wv[:, c, :],
                            start=(c == 0),
                            stop=(c == NCT - 1),
                        )
                    nc.vector.tensor_copy(
                        out=v_aug[:, tt, :, 0:64],
                        in_=psv[:, 0:256].rearrange("p (h e) -> p h e", h=HPC),
                    )

            # ------------- Phase B: attention, j-pairs outer -------------
            ytp_ctx = tc.tile_pool(name="ytp", bufs=1)
            ytp = ytp_ctx.__enter__()
            y_t = ytp.tile([128, 2, T], F32R)
            w2 = ytp.tile([128, 2, 1024], F32R)
            nc.sync.dma_start(w2, w2t_d.rearrange("o p e -> p o e"))
            with (
                tc.tile_pool(name="phB", bufs=1) as phb,
                tc.tile_pool(name="abp", bufs=2) as abp,
                tc.tile_pool(name="qp", bufs=3) as qp,
                tc.tile_pool(name="kp", bufs=3) as kp,
                tc.tile_pool(name="swp", bufs=2) as swp,
                tc.tile_pool(name="pp", bufs=7) as pp,
                tc.tile_pool(name="smalls", bufs=3) as smalls,
            ):
                cmask = phb.tile([128, 128], F32)
                nc.gpsimd.dma_start(cmask, cmask_d)
                for h in range(HPC):
                    abh = abp.tile([128, T], F32, tag="ab", name="abh")
                    nc.sync.dma_start(abh, ab_d[h])
                    qt = qp.tile([128, T], F32R, tag="qt", name="qt")
                    kt = kp.tile([128, T], F32R, tag="kt", name="kt")
                    # cross-partition moves through f32 scratch (shuffle
                    # can't write f32r; TensorTensor needs aligned bases)
                    musw = swp.tile([128, T], F32, tag="musw", name="musw")
                    for hb in range(2):
                        ts_ = slice(hb * 1024, hb * 1024 + 1024)
                        nc.vector.stream_shuffle(
                            musw[64:128, ts_], mu[h][0:64, ts_], idm
                        )
                        nc.vector.stream_shuffle(
                            musw[0:64, ts_], mu[h][64:128, ts_], idm
                        )
                        # shuffle-dependent halves on DVE (fast, critical
                        # path); independent halves on gpsimd (2x slower but
                        # off the critical path)
                        nc.vector.tensor_mul(
                            kt[0:64, ts_], musw[0:64, ts_], abh[0:64, ts_]
                        )
                        nc.gpsimd.tensor_mul(
                            kt[64:128, ts_], mu[h][64:128, ts_], abh[64:128, ts_]
                        )
                        nc.gpsimd.tensor_mul(
                            qt[0:64, ts_], mu[h][0:64, ts_], trig[0:64, ts_]
                        )
                        nc.vector.tensor_mul(
                            qt[64:128, ts_], musw[64:128, ts_], trig[64:128, ts_]
                        )

                    for jp in range(2):
                        j0, j1 = 2 * jp, 2 * jp + 1
                        yps = [
                            ps1.tile([128, 512], F32, tag="mm", name=f"ps_y{jj}")
                            for jj in range(2)
                        ]
                        imax = 4 * j1 + 3
                        for i in range(imax + 1):
                            jlo = i // 4  # lowest valid j for this tk tile
                            r = i % 4
                            jset = [j for j in (j0, j1) if j >= jlo]
                            # for the diagonal block (j == jlo) only columns
                            # >= 128*r can be causally valid: narrow the S
                            # matmul, exp and y matmul to that range; the
                            # skipped psum columns get no contribution from
                            # this tk tile, which is exactly correct.
                            sps = ps2.tile([128, 1024], F32, tag="s2", name="ps_s")
                            for j in jset:
                                o0 = (j - j0) * 512
                                lo = 128 * r if j == jlo else 0
                                nc.tensor.matmul(
                                    sps[:, o0 + lo : o0 + 512],
                                    lhsT=kt[:, i * 128 : (i + 1) * 128],
                                    rhs=qt[:, j * 512 + lo : (j + 1) * 512],
                                    start=True,
                                    stop=True,
                                )
                            p_sb = pp.tile([128, 1024], F32R, tag="p", name="p_sb")
                            c0 = (jset[0] - j0) * 512 + (
                                128 * r if jset[0] == jlo else 0
                            )
                            c1 = (jset[-1] - j0) * 512 + 512
                            nc.scalar.activation(
                                p_sb[:, c0:c1], sps[:, c0:c1], AF.Exp
                            )
                            if jlo in (j0, j1):
                                # mask only the 128-wide diagonal strip
                                boff = (jlo - j0) * 512 + 128 * r
                                nc.gpsimd.tensor_mul(
                                    p_sb[:, boff : boff + 128],
                                    p_sb[:, boff : boff + 128],
                                    cmask,
                                )
                            for j in jset:
                                o0 = (j - j0) * 512
                                lo = 128 * r if j == jlo else 0
                                nc.tensor.matmul(
                                    yps[j - j0][0:65, lo:512],
                                    lhsT=v_aug[:, i, h, :],
                                    rhs=p_sb[:, o0 + lo : o0 + 512],
                                    start=(i == 0),
                                    stop=(i == 4 * j + 3),
                                )
                        for jj, j in ((0, j0), (1, j1)):
                            # reciprocal directly on psum row 64 (same start
                            # partition for in and out keeps the ISA happy)
                            rc = smalls.tile([128, 512], F32R, tag="rc", name="rc")
                            with nc.allow_low_precision(
                                reason="f32r (~fp22) reciprocal of softmax denom"
                            ):
                                nc.vector.reciprocal(
                                    rc[64:65, :], yps[jj][64:65, :]
                                )
                            # broadcast across partitions via PE outer
                            # product: ones[1,128].T @ rc[1,512] -> psum
                            bps = ps1.tile([128, 512], F32, tag="mm", name="ps_bc")
                            nc.tensor.matmul(
                                bps,
                                lhsT=ones_t[64:65, :],
                                rhs=rc[64:65, :],
                                start=True,
                                stop=True,
                            )
                            bc = smalls.tile([128, 512], F32, tag="bc", name="bc")
                            nc.vector.tensor_copy(out=bc, in_=bps)
                            if h % 2 == 0:
                                nc.vector.tensor_mul(
                                    y_t[0:64, h // 2, j * 512 : (j + 1) * 512],
                                    yps[jj][0:64, :],
                                    bc[0:64, :],
                                )
                            else:
                                # odd heads land on partitions 64:128 of y_t
                                ysh = smalls.tile(
                                    [128, 512], F32, tag="ysh", name="ysh"
                                )
                                nc.vector.stream_shuffle(
                                    ysh[64:128, :], yps[jj][0:64, :], idm
                                )
                                nc.vector.tensor_mul(
                                    y_t[64:128, h // 2, j * 512 : (j + 1) * 512],
                                    ysh[64:128, :],
                                    bc[64:128, :],
                                )

            # ---------------- Phase C: output projection ----------------
            with tc.tile_pool(name="ostage", bufs=8) as ostage:
                for tt in range(NTT):
                    po = ps2.tile([128, 1024], F32, tag="s2", name="ps_o")
                    for ct in range(2):
                        for eh in range(2):
                            nc.tensor.matmul(
                                po[:, eh * 512 : eh * 512 + 512],
                                lhsT=y_t[:, ct, tt * 128 : (tt + 1) * 128],
                                rhs=w2[:, ct, eh * 512 : (eh + 1) * 512],
                                start=(ct == 0),
                                stop=(ct == 1),
                            )
                    ost = ostage.tile([128, 1024], F32, tag="o", name="ost")
                    if tt % 2 == 0:
                        nc.scalar.copy(ost, po)
                    else:
                        nc.vector.tensor_copy(out=ost, in_=po)
                    eng = nc.sync if tt % 2 == 0 else nc.gpsimd
                    eng.dma_start(out_d[tt], ost)

            ytp_ctx.__exit__(None, None, None)

    nc.compile()
    return nc


def make_inputs(x, w_attn, w_proj, delta):
    """Host-side prep: per-core input dicts (core = b*4 + g)."""
    x = np.asarray(x, dtype=np.float32)
    w_attn = np.asarray(w_attn, dtype=np.float32)
    w_proj = np.asarray(w_proj, dtype=np.float32)
    delta = np.asarray(delta, dtype=np.float32)

    inv_freq = 1.0 / (BASE ** (np.arange(D, dtype=np.float32) / D))
    t = np.arange(T, dtype=np.float32)
    freqs = t[:, None] * inv_freq[None, :]  # (T, D)
    scale = 1.0 / math.sqrt(D)
    cosTs = (np.cos(freqs).T * scale).astype(np.float32)  # (D, T)
    sinTs = (np.sin(freqs).T * scale).astype(np.float32)
    trig = np.concatenate([cosTs, sinTs], axis=0)  # (128, T)

    d = np.clip(delta, -2.0 * math.pi, 0.0)

    qw = w_attn[:C].reshape(H, D, C)
    kw = w_attn[C : 2 * C].reshape(H, D, C)
    vw = w_attn[2 * C :].reshape(H, D, C)

    # causal mask for the 128-wide diagonal strip: valid iff c >= tk
    tk = np.arange(128)[:, None]
    cc = np.arange(128)[None, :]
    cmask = (cc >= tk).astype(np.float32)

    in_maps = []
    for core in range(N_CORES):
        b, g = divmod(core, HPC)
        heads = range(HPC * g, HPC * g + HPC)

        xT = np.ascontiguousarray(x[b].T).reshape(NCT, 128, T)

        qk = np.stack(
            [np.concatenate([qw[h], kw[h]], axis=0) for h in heads], axis=0
        )  # (4, 128, C)
        wqk = np.ascontiguousarray(qk.transpose(2, 0, 1).reshape(C, 512)).reshape(
            NCT, 128, 512
        )
        wv = np.ascontiguousarray(
            vw[HPC * g : HPC * g + HPC].reshape(256, C).T
        ).reshape(NCT, 128, 256)
        w2t = np.ascontiguousarray(
            w_proj[:, 256 * g : 256 * (g + 1)].T
        ).reshape(2, 128, 1024)

        ab = np.stack(
            [
                np.concatenate(
                    [
                        np.cos(freqs + d[h][None, :]).T,
                        np.sin(freqs + d[h][None, :]).T,
                    ],
                    axis=0,
                ).astype(np.float32)
                for h in heads
            ],
            axis=0,
        )  # (4, 128, T)

        in_maps.append(
            {
                "xT": xT,
                "wqk": wqk,
                "wv": wv,
                "w2t": w2t,
                "trig": trig,
                "ab": ab,
                "cmask": cmask,
            }
        )
    return in_maps


_NC_CACHE = []


def _get_nc():
    if not _NC_CACHE:
        _NC_CACHE.append(build_module())
    return _NC_CACHE[0]


def kernel(x, w_attn, w_proj, delta, _trace=False):
    in_maps = make_inputs(x, w_attn, w_proj, delta)
    nc = _get_nc()
    res = None
    outs = None
    last_err = None
    for attempt in range(3):
        try:
            res = bass_utils.run_bass_kernel_spmd(
                nc, in_maps, core_ids=list(range(N_CORES)), trace=_trace
            )
            # jax results are async: force materialization inside the
            # retry so a transient NRT_EXEC_UNIT_UNRECOVERABLE (seen on
            # the first execution of a freshly-loaded NEFF) is caught
            outs = [
                np.asarray(r["out"]).reshape(T, C) for r in res.results
            ]
            break
        except Exception as e:
            last_err = e
            if "unrecoverable" not in str(e).lower() or attempt == 2:
                raise
            import time as _time

            _time.sleep(2.0)
    assert outs is not None, last_err
    if _trace:
        kernel.last_results = res
    full = np.zeros((B, T, C), dtype=np.float32)
    for core in range(N_CORES):
        full[core // HPC] += outs[core]
    return full



# revision 2
# speedup vs baseline: 1.0606x; 1.0606x over previous
"""Trainium2 Bass kernel v2 for causal self-attention with PoPE.

Reference (B=2, T=2048, C=1024, H=16, D=64):
  qkv = x @ w_attn.T ; mu = softplus(q|k)
  q_aug = mu_q * [cos(tw), sin(tw)] ; k_aug = mu_k * [cos(tw+d), sin(tw+d)]
  att = softmax_causal((q_aug . k_aug)/8) ; out = (att @ v) @ w_proj.T

Sharding: 8 cores = 2 batches x 4 head-groups (4 heads each); host sums
the 4 c_proj partials per batch.

Cost-model-driven design (CoreSim prices matmuls at out_free_cols x
cycles_per_row; fp8 DoubleRow = 0.5 cyc with K=256/pass; ACT = 0.833
ns/col; DVE f32 sbuf ops get 2x):
  - qk proj: fp8 DR, x_hi only (x and w*64 quantized host-side)
  - v  proj: fp8 DR 3-term (x_hi@wv_hi + x_hi@wv_lo + x_lo@wv_hi)
  - softplus: Exp(scale=1/64) in-place on psum + Ln(bias=1) -> mu f32
  - mu duplication across partition halves via SBUF->SBUF DMA
  - Qt: hi+lo fp8 residual pair in the DR j-dim; Kt: single fp8
    replicated via stride-0 j => S = Qt_exact . Kt_fp8 in 2 DR matmuls
  - causal mask folded into the S psum as fp8 matmuls (-240 additive
    lower-strict / full blocks), so exp of masked entries gives 0
  - P = exp(S/8 - 2.5) -> bf16 (ACT); y^T = P^T @ V_aug in bf16 per
    128-tq tile => y lands [tq, 64ch+denom] in psum; per-partition
    normalize (DVE recip + Pool bcast-mul)
  - y_t -> yT via PE transpose-mode matmuls; c_proj fp32r
Expected engine busy (per core): ACT ~94us (bottleneck), PE ~82us.
"""

import math
import os
import sys

import numpy as np
import ml_dtypes

for _p in ("/opt/trn_rl_repo",):
    if _p not in sys.path and os.path.isdir(_p):
        sys.path.insert(0, _p)

import concourse.tile as tile
from concourse import bacc
from concourse import mybir
from concourse import bass_utils

B, T, C = 2, 2048, 1024
H, D = 16, 64
BASE = 10000.0
N_CORES = 8
HPC = 4  # heads per core
NTT = 16  # t tiles of 128

F32 = mybir.dt.float32
F32R = mybir.dt.float32r
BF16 = mybir.dt.bfloat16
FP8 = mybir.dt.float8e4
AF = mybir.ActivationFunctionType
PM = mybir.MatmulPerfMode
OP = mybir.AluOpType
E4 = ml_dtypes.float8_e4m3
WSCALE = 64.0


def build_module():
    nc = bacc.Bacc(
        "TRN2", target_bir_lowering=False, debug=False, num_devices=N_CORES
    )

    xhi_d = nc.dram_tensor("xhi", (4, 128, 2, T), FP8, kind="ExternalInput").ap()
    xlo_d = nc.dram_tensor("xlo", (4, 128, 2, T), FP8, kind="ExternalInput").ap()
    wqk_d = nc.dram_tensor("wqk", (4, 128, 2, 512), FP8, kind="ExternalInput").ap()
    wvh_d = nc.dram_tensor("wvh", (4, 128, 2, 256), FP8, kind="ExternalInput").ap()
    wvl_d = nc.dram_tensor("wvl", (4, 128, 2, 256), FP8, kind="ExternalInput").ap()
    trig_d = nc.dram_tensor("trig", (128, T), BF16, kind="ExternalInput").ap()
    ab_d = nc.dram_tensor("ab", (HPC, 128, T), BF16, kind="ExternalInput").ap()
    mlow_d = nc.dram_tensor("mlow", (64, 2, 128), FP8, kind="ExternalInput").ap()
    mful_d = nc.dram_tensor("mful", (64, 2, 128), FP8, kind="ExternalInput").ap()
    idsp_d = nc.dram_tensor("idsp", (64, 2, 128), FP8, kind="ExternalInput").ap()
    i128_d = nc.dram_tensor("i128", (128, 128), BF16, kind="ExternalInput").ap()
    w2_d = nc.dram_tensor("w2", (2, 128, 1024), BF16, kind="ExternalInput").ap()
    out_d = nc.dram_tensor("out", (NTT, 128, 1024), F32, kind="ExternalOutput").ap()

    with tile.TileContext(nc) as tc:
        with (
            tc.tile_pool(name="persist", bufs=1) as persist,
            tc.tile_pool(name="qkpool", bufs=1) as qkpool,
            tc.tile_pool(name="ps2", bufs=2, space="PSUM") as ps2,
            tc.tile_pool(name="psy", bufs=2, space="PSUM") as psy,
            tc.tile_pool(name="psc", bufs=2, space="PSUM") as psc,
        ):
            # ---- persistent constants / outputs-in-sbuf ----
            mlow = persist.tile([64, 2, 128], FP8)
            mful = persist.tile([64, 2, 128], FP8)
            idsp = persist.tile([64, 2, 128], FP8)
            i128 = persist.tile([128, 128], BF16)
            trig = persist.tile([128, T], BF16)
            c_inv64 = persist.tile([128, 1], F32)
            c_inv8 = persist.tile([128, 1], F32)
            c_bias = persist.tile([128, 1], F32)
            nc.gpsimd.memset(c_inv64, 1.0 / WSCALE)
            nc.gpsimd.memset(c_inv8, 0.125)
            nc.gpsimd.memset(c_bias, -2.5)
            v_aug = persist.tile([128, NTT, HPC, 65], BF16)
            nc.vector.memset(
                v_aug.rearrange("p a b c -> p (a b) c")[:, :, 64:65], 1.0
            )
            # per-head S inputs (fp8): qt = [hi|lo] in DR j-dim; kt single
            qts = [qkpool.tile([128, 2, T], FP8, name=f"qt{h}") for h in range(HPC)]
            kts = [qkpool.tile([128, T], FP8, name=f"kt{h}") for h in range(HPC)]
            ktlos = [qkpool.tile([128, T], FP8, name=f"ktlo{h}") for h in range(HPC)]
            y_t = persist.tile([128, NTT, HPC, 64], BF16)
            yT = persist.tile([128, 2, T], BF16)
            w2 = persist.tile([128, 2, 1024], BF16)
            
            from concourse.tile_rust import add_dep_helper
            act_groups = {"E": [], "L": []}
            # ---------------- Phase A: QKV projection ----------------
            mupool_ctx = tc.tile_pool(name="mupool", bufs=1)
            mupool = mupool_ctx.__enter__()
            mu = [mupool.tile([128, T], BF16, name=f"mu{h}") for h in range(HPC)]

            a5_ctx = tc.tile_pool(name="a5", bufs=2)
            a5 = a5_ctx.__enter__()
            abp_ctx = tc.tile_pool(name="abp", bufs=2)
            abp = abp_ctx.__enter__()
            phb_ctx = tc.tile_pool(name="phB", bufs=17)
            phb = phb_ctx.__enter__()
            ost_ctx = tc.tile_pool(name="ostage", bufs=3)
            ostage = ost_ctx.__enter__()
            rn_ctx = tc.tile_pool(name="rn", bufs=2)
            rnp = rn_ctx.__enter__()
            pha_ctx = tc.tile_pool(name="phA", bufs=1)
            pha = pha_ctx.__enter__()
            xhi = pha.tile([128, 4, 2, T], FP8)
            xlo = pha.tile([128, 4, 2, T], FP8)
            wqk = pha.tile([128, 4, 2, 512], FP8)
            wvh = pha.tile([128, 4, 2, 256], FP8)
            wvl = pha.tile([128, 4, 2, 256], FP8)
            nc.sync.dma_start(wqk, wqk_d.rearrange("o p j e -> p o j e"))
            engs = [nc.sync, nc.gpsimd, nc.sync, nc.gpsimd]
            for th in range(2):
                tsl = slice(th * 1024, th * 1024 + 1024)
                for cc in range(4):
                    engs[cc].dma_start(xhi[:, cc, :, tsl], xhi_d[cc][:, :, tsl])
            nc.sync.dma_start(wvh, wvh_d.rearrange("o p j e -> p o j e"))
            nc.sync.dma_start(wvl, wvl_d.rearrange("o p j e -> p o j e"))
            for th in range(2):
                tsl = slice(th * 1024, th * 1024 + 1024)
                for cc in range(4):
                    engs[(cc + 1) % 4].dma_start(
                        xlo[:, cc, :, tsl], xlo_d[cc][:, :, tsl]
                    )
            nc.gpsimd.dma_start(trig, trig_d)
            nc.gpsimd.dma_start(mlow, mlow_d)
            nc.gpsimd.dma_start(mful, mful_d)
            nc.gpsimd.dma_start(idsp, idsp_d)
            nc.gpsimd.dma_start(i128, i128_d)
            nc.gpsimd.dma_start(w2, w2_d.rearrange("o p e -> p o e"))

            # qk: per (head, 1024-block): psum [128,1024], 4 DR matmuls over cc
            # v: out[t, 256] per t-tile, 4 tiles per psum; 3-term fp8 DR.
            # Interleave qk and v allocations so PE has v work while ACT
            # drains softplus.
            def emit_qk(h, tb):
                ts_ = slice(tb * 1024, tb * 1024 + 1024)
                qk_ps = ps2.tile([128, 1024], F32, tag="s2", name="qk_ps")
                for half in range(2):
                    hs = slice(tb * 1024 + half * 512, tb * 1024 + half * 512 + 512)
                    for cc in range(4):
                        nc.tensor.matmul(
                            qk_ps[:, half * 512 : half * 512 + 512],
                            lhsT=wqk[:, cc, :, h * 128 : (h + 1) * 128],
                            rhs=xhi[:, cc, :, hs],
                            start=(cc == 0),
                            stop=(cc == 3),
                            perf_mode=PM.DoubleRow,
                        )
                # softplus = ln(exp(q/64)+1): exp -> mu (bf16), ln batched
                # later so the ACT table doesn't ping-pong between sets
                act_groups["E"].append(
                    nc.scalar.activation(mu[h][:, ts_], qk_ps, AF.Exp, scale=c_inv64)
                )

            def emit_v(tq):  # tq = t-pair index 0..7
                v_ps = psc.tile([128, 2, 256], F32, tag="pc", name="v_ps")
                for s in range(2):
                    tt = 2 * tq + s
                    tsl = slice(tt * 128, (tt + 1) * 128)
                    terms = ((xhi, wvh), (xhi, wvl), (xlo, wvh))
                    n = 0
                    for cc in range(4):
                        for (xx, ww) in terms:
                            nc.tensor.matmul(
                                v_ps[:, s, :],
                                lhsT=xx[:, cc, :, tsl],
                                rhs=ww[:, cc],
                                start=(n == 0),
                                stop=(n == 11),
                                perf_mode=PM.DoubleRow,
                            )
                            n += 1
                nc.vector.tensor_copy(
                    out=v_aug[:, 2 * tq : 2 * tq + 2, :, 0:64],
                    in_=v_ps.rearrange("p s (h e) -> p s h e", h=HPC),
                )

            for h in range(HPC):
                emit_qk(h, 0)
                emit_qk(h, 1)
            for tq in range(8):
                emit_v(tq)
            for h in range(HPC):
                for tb in range(2):
                    ts_ = slice(tb * 1024, tb * 1024 + 1024)
                    ln = nc.scalar.activation(
                        mu[h][:, ts_], mu[h][:, ts_], AF.Ln, bias=1.0
                    )
                    add_dep_helper(ln.ins, act_groups["E"][-1].ins, sync=False,
                                   reason="group phase-A Lns after Exps (ACT table)")
                    act_groups["L"].append(ln)

            pha_ctx.__exit__(None, None, None)

            # ------- Phase A.5: Qt/Kt formation (per head) -------
            for h in range(HPC):
                abh = abp.tile([128, T], BF16, tag="ab", name="abh")
                nc.sync.dma_start(abh, ab_d[h])
                # musw = [mu_k ; mu_q] (swapped halves) via DVE shuffles;
                # processed in 1024-col halves, high half first (jb runs
                # descending, so high-t qt cols are needed first; kt pair 0
                # needs low-t keys first -> kt low half first)
                musw = a5.tile([128, T], BF16, tag="musw", name="musw")
                qtf = a5.tile([128, T], BF16, tag="qtf", name="qtf")
                ktf = a5.tile([128, T], BF16, tag="ktf", name="ktf")
                idm = list(range(32))
                qeng = nc.vector if h == 0 else nc.gpsimd
                for hb in (0, 1) if h == 0 else ((1, 0)):
                    # kt chain on Pool (low keys first: S pair 0 reads them)
                    ts_ = slice(hb * 1024, hb * 1024 + 1024)
                    nc.vector.stream_shuffle(
                        musw[0:64, ts_], mu[h][64:128, ts_], idm
                    )
                    nc.gpsimd.tensor_mul(
                        ktf[0:64, ts_], musw[0:64, ts_], abh[0:64, ts_]
                    )
                    nc.gpsimd.tensor_mul(
                        ktf[64:128, ts_], mu[h][64:128, ts_], abh[64:128, ts_]
                    )
                    nc.gpsimd.tensor_copy(out=kts[h][:, ts_], in_=ktf[:, ts_])
                    nc.vector.scalar_tensor_tensor(
                        ktlos[h][:, ts_], kts[h][:, ts_], -1.0, ktf[:, ts_],
                        OP.mult, OP.add
                    )
                for hb in (1, 0):
                    # qt chain on DVE (h0: incl. fp8 ops, parallel with Pool)
                    ts_ = slice(hb * 1024, hb * 1024 + 1024)
                    nc.vector.stream_shuffle(
                        musw[64:128, ts_], mu[h][0:64, ts_], idm
                    )
                    nc.vector.tensor_mul(
                        qtf[0:64, ts_], mu[h][0:64, ts_], trig[0:64, ts_]
                    )
                    nc.vector.tensor_mul(
                        qtf[64:128, ts_], musw[64:128, ts_], trig[64:128, ts_]
                    )
                    qeng.tensor_copy(
                        out=qts[h][:, 0, ts_], in_=qtf[:, ts_]
                    )
                    nc.vector.scalar_tensor_tensor(
                        qts[h][:, 1, ts_], qts[h][:, 0, ts_], -1.0, qtf[:, ts_],
                        OP.mult, OP.add
                    )

            # ------- Phase B: attention + transpose + c_proj, per tq block ----
            proj_pending = []
            emit_proj_fns = {}
            for jb in (3, 2, 1, 0):
                cols = slice(jb * 512, (jb + 1) * 512)
                def emit_y(h, p_sbs, y_ps):
                    for q in range(4):
                        jt = 4 * jb + q
                        for kt in range(jt + 1):
                            nc.tensor.matmul(
                                y_ps[:, q, :],
                                lhsT=p_sbs[kt // 2][:, kt % 2, q * 128 : (q + 1) * 128],
                                rhs=v_aug[:, kt, h, :],
                                start=(kt == 0),
                                stop=(kt == jt),
                            )
                    rn = rnp.tile([128, 4], F32, tag="rn", name="rn")
                    nc.vector.reciprocal(
                        rn, y_ps[:, :, 64:65].rearrange("p a b -> p (a b)")
                    )
                    rnb = rn.rearrange("p (a b) -> p a b", b=1).broadcast_to(
                        [128, 4, 64]
                    )
                    nc.vector.tensor_mul(
                        y_t[:, 4 * jb : 4 * jb + 4, h, :], y_ps[:, :, 0:64], rnb
                    )

                pending = None
                for h in range(HPC):
                    y_ps = psy.tile([128, 4, 65], F32, tag="y1", name="y_ps")
                    p_sbs = []
                    for p in range(2 * jb + 2):
                        c0 = max(0, 256 * p - 512 * jb)
                        csl = slice(jb * 512 + c0, (jb + 1) * 512)
                        sps = ps2.tile([128, 2, 512], F32, tag="s2", name="sps")
                        for s in range(2):
                            kt = 2 * p + s
                            ksl = slice(kt * 128, (kt + 1) * 128)
                            ktb = kts[h][:, ksl].rearrange(
                                "p (j m) -> p j m", j=1
                            ).broadcast_to([128, 2, 128])
                            ktlob = ktlos[h][:, ksl].rearrange(
                                "p (j m) -> p j m", j=1
                            ).broadcast_to([128, 2, 128])
                            nc.tensor.matmul(
                                sps[:, s, c0:512],
                                lhsT=ktb,
                                rhs=qts[h][:, :, csl],
                                start=True,
                                stop=False,
                                perf_mode=PM.DoubleRow,
                            )
                            d0 = 128 * kt - 512 * jb  # diag block col offset
                            if 0 <= d0 < 512:
                                nc.tensor.matmul(
                                    sps[:, s, d0 : d0 + 128],
                                    lhsT=mlow,
                                    rhs=idsp,
                                    start=False,
                                    stop=False,
                                    perf_mode=PM.DoubleRow,
                                )
                                if d0 > c0:
                                    # fully-masked strip left of the diagonal
                                    nc.tensor.matmul(
                                        sps[:, s, c0:d0],
                                        lhsT=mful,
                                        rhs=idsp[:, :, 0 : d0 - c0],
                                        start=False,
                                        stop=False,
                                        perf_mode=PM.DoubleRow,
                                    )
                            nc.tensor.matmul(
                                sps[:, s, c0:512],
                                lhsT=ktlob,
                                rhs=qts[h][:, :, csl],
                                start=False,
                                stop=True,
                                perf_mode=PM.DoubleRow,
                            )
                        p_sb = phb.tile([128, 2, 512], BF16, tag="p", name="p_sb")
                        ex = nc.scalar.activation(
                            p_sb[:, :, c0:512],
                            sps[:, :, c0:512],
                            AF.Exp,
                            scale=c_inv8,
                            bias=c_bias,
                        )
                        if act_groups["L"]:
                            add_dep_helper(ex.ins, act_groups["L"][-1].ins, sync=False,
                                           reason="phase-B exps after phase-A Lns (ACT table)")
                            act_groups["L"] = []
                        p_sbs.append(p_sb)
                    if h == 0 and proj_pending:
                        emit_proj_fns[proj_pending.pop()]()
                    if pending is not None:
                        emit_y(*pending)
                    pending = (h, p_sbs, y_ps)
                emit_y(*pending)
                # transpose 4 t-tiles + c_proj + stage + dma out (deferred:
                # emitted after the next jb's first head S/exp so PE's
                # in-order queue doesn't stall ACT at jb boundaries)
                def emit_proj(jb=jb):
                  for q in range(4):
                    tt = 4 * jb + q
                    tsl = slice(tt * 128, (tt + 1) * 128)
                    pc = psc.tile([128, 2, 128], BF16, tag="pc", name="pc")
                    for cc in range(2):
                        nc.tensor.matmul(
                            pc[:, cc, :],
                            lhsT=y_t[:, tt, 2 * cc : 2 * cc + 2, :],
                            rhs=i128,
                            start=True,
                            stop=True,
                            is_transpose=True,
                        )
                    nc.vector.tensor_copy(out=yT[:, :, tsl], in_=pc)
                  for q in range(4):
                    tt = 4 * jb + q
                    tsl = slice(tt * 128, (tt + 1) * 128)
                    ost = ostage.tile([128, 1024], F32, tag="o", name="ost")
                    for eh in range(2):
                        po = psc.tile([128, 512], F32, tag="pc", name="po")
                        for cc in range(2):
                            nc.tensor.matmul(
                                po,
                                lhsT=yT[:, cc, tsl],
                                rhs=w2[:, cc, eh * 512 : (eh + 1) * 512],
                                start=(cc == 0),
                                stop=(cc == 1),
                            )
                        nc.vector.tensor_copy(
                            out=ost[:, eh * 512 : (eh + 1) * 512], in_=po
                        )
                    deng = nc.sync if q % 2 == 0 else nc.gpsimd
                    deng.dma_start(out_d[tt], ost)
                emit_proj_fns[jb] = emit_proj
                proj_pending.append(jb)
            while proj_pending:
                emit_proj_fns[proj_pending.pop()]()
            rn_ctx.__exit__(None, None, None)
            ost_ctx.__exit__(None, None, None)
            phb_ctx.__exit__(None, None, None)
            abp_ctx.__exit__(None, None, None)
            a5_ctx.__exit__(None, None, None)
            mupool_ctx.__exit__(None, None, None)

    nc.compile()
    return nc


def make_inputs(x, w_attn, w_proj, delta):
    """Host-side prep: per-core input dicts (core = b*4 + g)."""
    x = np.asarray(x, dtype=np.float32)
    w_attn = np.asarray(w_attn, dtype=np.float32)
    w_proj = np.asarray(w_proj, dtype=np.float32)
    delta = np.asarray(delta, dtype=np.float32)

    inv_freq = 1.0 / (BASE ** (np.arange(D, dtype=np.float32) / D))
    t = np.arange(T, dtype=np.float64)
    freqs = (t[:, None] * inv_freq[None, :].astype(np.float64)).astype(np.float32)
    cosT = np.cos(freqs).T.astype(np.float32)  # (D, T)
    sinT = np.sin(freqs).T.astype(np.float32)
    trig = np.concatenate([cosT, sinT], axis=0).astype(ml_dtypes.bfloat16)
    d = np.clip(delta, -2.0 * math.pi, 0.0)

    # fp8 split of x per batch: [4cc, 128p, 2j, T] with c = cc*256+j*128+p
    def to_dr(mat):  # mat (T, C) -> (4, 128, 2, T)
        m = mat.T.reshape(4, 2, 128, T)  # (cc, j, p, t)
        return np.ascontiguousarray(m.transpose(0, 2, 1, 3))

    xw = [None] * B
    for b in range(B):
        xb = x[b]  # (T, C)
        x_hi = xb.astype(E4)
        x_lo = (xb - x_hi.astype(np.float32)).astype(E4)
        xw[b] = (to_dr(x_hi.astype(np.float32)).astype(E4),
                 to_dr(x_lo.astype(np.float32)).astype(E4))

    qw = w_attn[:C].reshape(H, D, C)
    kw = w_attn[C : 2 * C].reshape(H, D, C)
    vw = w_attn[2 * C :].reshape(H, D, C)

    # mask constants
    pp, jj, tk = np.meshgrid(
        np.arange(64), np.arange(2), np.arange(128), indexing="ij"
    )
    f = jj * 64 + pp
    mlow = np.where(tk > f, -240.0, 0.0).astype(E4)
    mful = np.full((64, 2, 128), -240.0, dtype=np.float32).astype(E4)
    idsp = (tk == f).astype(np.float32).astype(E4)
    i128 = np.eye(128, dtype=np.float32).astype(ml_dtypes.bfloat16)

    in_maps = []
    for core in range(N_CORES):
        b, g = divmod(core, HPC)
        heads = list(range(HPC * g, HPC * g + HPC))

        # wqk: (4cc, 128p, 2j, 512): col = h*128 + r; r<64 q_d else k_d
        wqk_full = np.empty((C, 512), dtype=np.float32)  # (c, col)
        for hi_, hg in enumerate(heads):
            wqk_full[:, hi_ * 128 : hi_ * 128 + 64] = qw[hg].T * WSCALE
            wqk_full[:, hi_ * 128 + 64 : hi_ * 128 + 128] = kw[hg].T * WSCALE
        wqk8 = wqk_full.astype(E4)
        wqk_dr = np.ascontiguousarray(
            wqk8.reshape(4, 2, 128, 512).transpose(0, 2, 1, 3)
        )

        wv_full = (
            vw[HPC * g : HPC * g + HPC].reshape(256, C).T * WSCALE
        )  # (c, 256)
        wv_hi = wv_full.astype(E4)
        wv_lo = (wv_full - wv_hi.astype(np.float32)).astype(E4)
        wvh_dr = np.ascontiguousarray(
            wv_hi.reshape(4, 2, 128, 256).transpose(0, 2, 1, 3)
        )
        wvl_dr = np.ascontiguousarray(
            wv_lo.reshape(4, 2, 128, 256).transpose(0, 2, 1, 3)
        )

        ab = np.stack(
            [
                np.concatenate(
                    [
                        np.cos(freqs + d[hg][None, :]).T,
                        np.sin(freqs + d[hg][None, :]).T,
                    ],
                    axis=0,
                ).astype(ml_dtypes.bfloat16)
                for hg in heads
            ],
            axis=0,
        )  # (4, 128, T)

        # w2: (2cc, 128p, 1024e): channel c_local = cc*128 + p of this group's
        # 256 y channels; y channel (h_local, dd) flattened h_local*64+dd
        w2g = w_proj[:, 256 * g : 256 * (g + 1)]  # (e, 256)
        w2_dr = np.ascontiguousarray((w2g.T / WSCALE).reshape(2, 128, 1024)).astype(ml_dtypes.bfloat16)

        in_maps.append(
            {
                "xhi": xw[b][0],
                "xlo": xw[b][1],
                "wqk": wqk_dr,
                "wvh": wvh_dr,
                "wvl": wvl_dr,
                "trig": trig,
                "ab": ab,
                "mlow": mlow,
                "mful": mful,
                "idsp": idsp,
                "i128": i128,
                "w2": w2_dr,
            }
        )
    return in_maps


_NC_CACHE = []


def _get_nc():
    if not _NC_CACHE:
        _NC_CACHE.append(build_module())
    return _NC_CACHE[0]


def kernel(x, w_attn, w_proj, delta, _trace=False):
    in_maps = make_inputs(x, w_attn, w_proj, delta)
    nc = _get_nc()
    res = None
    outs = None
    last_err = None
    for attempt in range(3):
        try:
            res = bass_utils.run_bass_kernel_spmd(
                nc, in_maps, core_ids=list(range(N_CORES)), trace=_trace
            )
            outs = [np.asarray(r["out"]).reshape(T, C) for r in res.results]
            break
        except Exception as e:
            last_err = e
            if "unrecoverable" not in str(e).lower() or attempt == 2:
                raise
            import time as _time

            _time.sleep(2.0)
    assert outs is not None, last_err
    if _trace:
        kernel.last_results = res
    full = np.zeros((B, T, C), dtype=np.float32)
    for core in range(N_CORES):
        full[core // HPC] += outs[core]
    return full


# revision 3
# speedup vs baseline: 1.0620x; 1.0014x over previous
"""Trainium2 Bass kernel v2 for causal self-attention with PoPE.

Reference (B=2, T=2048, C=1024, H=16, D=64):
  qkv = x @ w_attn.T ; mu = softplus(q|k)
  q_aug = mu_q * [cos(tw), sin(tw)] ; k_aug = mu_k * [cos(tw+d), sin(tw+d)]
  att = softmax_causal((q_aug . k_aug)/8) ; out = (att @ v) @ w_proj.T

Sharding: 8 cores = 2 batches x 4 head-groups (4 heads each); host sums
the 4 c_proj partials per batch.

Cost-model-driven design (CoreSim prices matmuls at out_free_cols x
cycles_per_row; fp8 DoubleRow = 0.5 cyc with K=256/pass; ACT = 0.833
ns/col; DVE f32 sbuf ops get 2x):
  - qk proj: fp8 DR, x_hi only (x and w*64 quantized host-side)
  - v  proj: fp8 DR 3-term (x_hi@wv_hi + x_hi@wv_lo + x_lo@wv_hi)
  - softplus: Exp(scale=1/64) in-place on psum + Ln(bias=1) -> mu f32
  - mu duplication across partition halves via SBUF->SBUF DMA
  - Qt: hi+lo fp8 residual pair in the DR j-dim; Kt: single fp8
    replicated via stride-0 j => S = Qt_exact . Kt_fp8 in 2 DR matmuls
  - causal mask folded into the S psum as fp8 matmuls (-240 additive
    lower-strict / full blocks), so exp of masked entries gives 0
  - P = exp(S/8 - 2.5) -> bf16 (ACT); y^T = P^T @ V_aug in bf16 per
    128-tq tile => y lands [tq, 64ch+denom] in psum; per-partition
    normalize (DVE recip + Pool bcast-mul)
  - y_t -> yT via PE transpose-mode matmuls; c_proj fp32r
Expected engine busy (per core): ACT ~94us (bottleneck), PE ~82us.
"""

import math
import os
import sys

import numpy as np
import ml_dtypes

for _p in ("/opt/trn_rl_repo",):
    if _p not in sys.path and os.path.isdir(_p):
        sys.path.insert(0, _p)

import concourse.tile as tile
from concourse import bacc
from concourse import mybir
from concourse import bass_utils

B, T, C = 2, 2048, 1024
H, D = 16, 64
BASE = 10000.0
N_CORES = 8
HPC = 4  # heads per core
NTT = 16  # t tiles of 128

F32 = mybir.dt.float32
F32R = mybir.dt.float32r
BF16 = mybir.dt.bfloat16
FP8 = mybir.dt.float8e4
AF = mybir.ActivationFunctionType
PM = mybir.MatmulPerfMode
OP = mybir.AluOpType
E4 = ml_dtypes.float8_e4m3
WSCALE = 64.0


def build_module():
    nc = bacc.Bacc(
        "TRN2", target_bir_lowering=False, debug=False, num_devices=N_CORES
    )

    xhi_d = nc.dram_tensor("xhi", (4, 128, 2, T), FP8, kind="ExternalInput").ap()
    xlo_d = nc.dram_tensor("xlo", (4, 128, 2, T), FP8, kind="ExternalInput").ap()
    wqk_d = nc.dram_tensor("wqk", (4, 128, 2, 512), FP8, kind="ExternalInput").ap()
    wvh_d = nc.dram_tensor("wvh", (4, 128, 2, 256), FP8, kind="ExternalInput").ap()
    wvl_d = nc.dram_tensor("wvl", (4, 128, 2, 256), FP8, kind="ExternalInput").ap()
    trig_d = nc.dram_tensor("trig", (128, T), BF16, kind="ExternalInput").ap()
    ab_d = nc.dram_tensor("ab", (HPC, 128, T), BF16, kind="ExternalInput").ap()
    mlow_d = nc.dram_tensor("mlow", (64, 2, 128), FP8, kind="ExternalInput").ap()
    mful_d = nc.dram_tensor("mful", (64, 2, 128), FP8, kind="ExternalInput").ap()
    idsp_d = nc.dram_tensor("idsp", (64, 2, 128), FP8, kind="ExternalInput").ap()
    i128_d = nc.dram_tensor("i128", (128, 128), BF16, kind="ExternalInput").ap()
    w2_d = nc.dram_tensor("w2", (2, 128, 1024), BF16, kind="ExternalInput").ap()
    out_d = nc.dram_tensor("out", (NTT, 128, 1024), F32, kind="ExternalOutput").ap()

    with tile.TileContext(nc) as tc:
        with (
            tc.tile_pool(name="persist", bufs=1) as persist,
            tc.tile_pool(name="qkpool", bufs=1) as qkpool,
            tc.tile_pool(name="ps2", bufs=2, space="PSUM") as ps2,
            tc.tile_pool(name="psy", bufs=2, space="PSUM") as psy,
            tc.tile_pool(name="psc", bufs=2, space="PSUM") as psc,
        ):
            # ---- persistent constants / outputs-in-sbuf ----
            mlow = persist.tile([64, 2, 128], FP8)
            mful = persist.tile([64, 2, 128], FP8)
            idsp = persist.tile([64, 2, 128], FP8)
            i128 = persist.tile([128, 128], BF16)
            trig = persist.tile([128, T], BF16)
            c_inv64 = persist.tile([128, 1], F32)
            c_inv8 = persist.tile([128, 1], F32)
            c_bias = persist.tile([128, 1], F32)
            nc.gpsimd.memset(c_inv64, 1.0 / WSCALE)
            nc.gpsimd.memset(c_inv8, 0.125)
            nc.gpsimd.memset(c_bias, -2.5)
            v_aug = persist.tile([128, NTT, HPC, 65], BF16)
            nc.vector.memset(
                v_aug.rearrange("p a b c -> p (a b) c")[:, :, 64:65], 1.0
            )
            # per-head S inputs (fp8): qt = [hi|lo] in DR j-dim; kt single
            qts = [qkpool.tile([128, 2, T], FP8, name=f"qt{h}") for h in range(HPC)]
            kts = [qkpool.tile([128, T], FP8, name=f"kt{h}") for h in range(HPC)]
            ktlos = [qkpool.tile([128, T], FP8, name=f"ktlo{h}") for h in range(HPC)]
            y_t = persist.tile([128, NTT, HPC, 64], BF16)
            yT = persist.tile([128, 2, T], BF16)
            w2 = persist.tile([128, 2, 1024], BF16)
            
            from concourse.tile_rust import add_dep_helper
            act_groups = {"E": [], "L": []}
            # ---------------- Phase A: QKV projection ----------------
            mupool_ctx = tc.tile_pool(name="mupool", bufs=1)
            mupool = mupool_ctx.__enter__()
            mu = [mupool.tile([128, T], BF16, name=f"mu{h}") for h in range(HPC)]

            a5_ctx = tc.tile_pool(name="a5", bufs=2)
            a5 = a5_ctx.__enter__()
            abp_ctx = tc.tile_pool(name="abp", bufs=2)
            abp = abp_ctx.__enter__()
            phb_ctx = tc.tile_pool(name="phB", bufs=17)
            phb = phb_ctx.__enter__()
            ost_ctx = tc.tile_pool(name="ostage", bufs=3)
            ostage = ost_ctx.__enter__()
            rn_ctx = tc.tile_pool(name="rn", bufs=2)
            rnp = rn_ctx.__enter__()
            pha_ctx = tc.tile_pool(name="phA", bufs=1)
            pha = pha_ctx.__enter__()
            xhi = pha.tile([128, 4, 2, T], FP8)
            xlo = pha.tile([128, 4, 2, T], FP8)
            wqk = pha.tile([128, 4, 2, 512], FP8)
            wvh = pha.tile([128, 4, 2, 256], FP8)
            wvl = pha.tile([128, 4, 2, 256], FP8)
            nc.sync.dma_start(wqk, wqk_d.rearrange("o p j e -> p o j e"))
            engs = [nc.sync, nc.gpsimd, nc.sync, nc.gpsimd]
            for th in range(2):
                tsl = slice(th * 1024, th * 1024 + 1024)
                for cc in range(4):
                    engs[cc].dma_start(xhi[:, cc, :, tsl], xhi_d[cc][:, :, tsl])
            nc.sync.dma_start(wvh, wvh_d.rearrange("o p j e -> p o j e"))
            nc.sync.dma_start(wvl, wvl_d.rearrange("o p j e -> p o j e"))
            for th in range(2):
                tsl = slice(th * 1024, th * 1024 + 1024)
                for cc in range(4):
                    engs[(cc + 1) % 4].dma_start(
                        xlo[:, cc, :, tsl], xlo_d[cc][:, :, tsl]
                    )
            nc.gpsimd.dma_start(trig, trig_d)
            nc.gpsimd.dma_start(mlow, mlow_d)
            nc.gpsimd.dma_start(mful, mful_d)
            nc.gpsimd.dma_start(idsp, idsp_d)
            nc.gpsimd.dma_start(i128, i128_d)
            nc.gpsimd.dma_start(w2, w2_d.rearrange("o p e -> p o e"))

            # qk: per (head, 1024-block): psum [128,1024], 4 DR matmuls over cc
            # v: out[t, 256] per t-tile, 4 tiles per psum; 3-term fp8 DR.
            # Interleave qk and v allocations so PE has v work while ACT
            # drains softplus.
            def emit_qk(h, tb):
                ts_ = slice(tb * 1024, tb * 1024 + 1024)
                qk_ps = ps2.tile([128, 1024], F32, tag="s2", name="qk_ps")
                for half in range(2):
                    hs = slice(tb * 1024 + half * 512, tb * 1024 + half * 512 + 512)
                    for cc in range(4):
                        nc.tensor.matmul(
                            qk_ps[:, half * 512 : half * 512 + 512],
                            lhsT=wqk[:, cc, :, h * 128 : (h + 1) * 128],
                            rhs=xhi[:, cc, :, hs],
                            start=(cc == 0),
                            stop=(cc == 3),
                            perf_mode=PM.DoubleRow,
                        )
                # softplus = ln(exp(q/64)+1): exp -> mu (bf16), ln batched
                # later so the ACT table doesn't ping-pong between sets
                act_groups["E"].append(
                    nc.scalar.activation(mu[h][:, ts_], qk_ps, AF.Exp, scale=c_inv64)
                )

            def emit_v(tq):  # tq = t-pair index 0..7
                v_ps = psc.tile([128, 2, 256], F32, tag="pc", name="v_ps")
                for s in range(2):
                    tt = 2 * tq + s
                    tsl = slice(tt * 128, (tt + 1) * 128)
                    terms = ((xhi, wvh), (xhi, wvl), (xlo, wvh))
                    n = 0
                    for cc in range(4):
                        for (xx, ww) in terms:
                            nc.tensor.matmul(
                                v_ps[:, s, :],
                                lhsT=xx[:, cc, :, tsl],
                                rhs=ww[:, cc],
                                start=(n == 0),
                                stop=(n == 11),
                                perf_mode=PM.DoubleRow,
                            )
                            n += 1
                nc.vector.tensor_copy(
                    out=v_aug[:, 2 * tq : 2 * tq + 2, :, 0:64],
                    in_=v_ps.rearrange("p s (h e) -> p s h e", h=HPC),
                )

            for h in range(HPC):
                emit_qk(h, 0)
                emit_qk(h, 1)
            for tq in range(8):
                emit_v(tq)
            for h in range(HPC):
                ln = nc.scalar.activation(mu[h], mu[h], AF.Ln, bias=1.0)
                add_dep_helper(ln.ins, act_groups["E"][-1].ins, sync=False,
                               reason="group phase-A Lns after Exps (ACT table)")
                act_groups["L"].append(ln)

            pha_ctx.__exit__(None, None, None)

            # ------- Phase A.5: Qt/Kt formation (per head) -------
            for h in range(HPC):
                abh = abp.tile([128, T], BF16, tag="ab", name="abh")
                nc.sync.dma_start(abh, ab_d[h])
                # musw = [mu_k ; mu_q] (swapped halves) via DVE shuffles;
                # processed in 1024-col halves, high half first (jb runs
                # descending, so high-t qt cols are needed first; kt pair 0
                # needs low-t keys first -> kt low half first)
                musw = a5.tile([128, T], BF16, tag="musw", name="musw")
                qtf = a5.tile([128, T], BF16, tag="qtf", name="qtf")
                ktf = a5.tile([128, T], BF16, tag="ktf", name="ktf")
                idm = list(range(32))
                qeng = nc.vector if h == 0 else nc.gpsimd
                for hb in (0,) if h == 0 else ((1, 0)):
                    # kt chain on Pool (low keys first: S pair 0 reads them)
                    ts_ = slice(hb * 1024, hb * 1024 + 1024)
                    nc.vector.stream_shuffle(
                        musw[0:64, ts_], mu[h][64:128, ts_], idm
                    )
                    nc.gpsimd.tensor_mul(
                        ktf[0:64, ts_], musw[0:64, ts_], abh[0:64, ts_]
                    )
                    nc.gpsimd.tensor_mul(
                        ktf[64:128, ts_], mu[h][64:128, ts_], abh[64:128, ts_]
                    )
                    nc.gpsimd.tensor_copy(out=kts[h][:, ts_], in_=ktf[:, ts_])
                    nc.gpsimd.tensor_sub(
                        ktlos[h][:, ts_], ktf[:, ts_], kts[h][:, ts_]
                    )
                for hb in ((1, 0) if h != 0 else (1,)):
                    # qt chain on DVE (h0: incl. fp8 ops, parallel with Pool)
                    ts_ = slice(hb * 1024, hb * 1024 + 1024)
                    nc.vector.stream_shuffle(
                        musw[64:128, ts_], mu[h][0:64, ts_], idm
                    )
                    nc.vector.tensor_mul(
                        qtf[0:64, ts_], mu[h][0:64, ts_], trig[0:64, ts_]
                    )
                    nc.vector.tensor_mul(
                        qtf[64:128, ts_], musw[64:128, ts_], trig[64:128, ts_]
                    )
                    qeng.tensor_copy(
                        out=qts[h][:, 0, ts_], in_=qtf[:, ts_]
                    )
                    qeng.tensor_sub(
                        qts[h][:, 1, ts_], qtf[:, ts_], qts[h][:, 0, ts_]
                    )
                if h == 0:
                    for ts_ in (slice(1024, 2048), slice(0, 1024)):
                        nc.vector.stream_shuffle(
                            musw[0:64, ts_] if ts_.start == 1024 else musw[64:128, ts_],
                            mu[h][64:128, ts_] if ts_.start == 1024 else mu[h][0:64, ts_],
                            idm,
                        )
                    ts_ = slice(1024, 2048)
                    nc.gpsimd.tensor_mul(
                        ktf[0:64, ts_], musw[0:64, ts_], abh[0:64, ts_]
                    )
                    nc.gpsimd.tensor_mul(
                        ktf[64:128, ts_], mu[h][64:128, ts_], abh[64:128, ts_]
                    )
                    nc.gpsimd.tensor_copy(out=kts[h][:, ts_], in_=ktf[:, ts_])
                    nc.gpsimd.tensor_sub(
                        ktlos[h][:, ts_], ktf[:, ts_], kts[h][:, ts_]
                    )
                    ts_ = slice(0, 1024)
                    nc.vector.tensor_mul(
                        qtf[0:64, ts_], mu[h][0:64, ts_], trig[0:64, ts_]
                    )
                    nc.vector.tensor_mul(
                        qtf[64:128, ts_], musw[64:128, ts_], trig[64:128, ts_]
                    )
                    nc.vector.tensor_copy(out=qts[h][:, 0, ts_], in_=qtf[:, ts_])
                    nc.vector.tensor_sub(
                        qts[h][:, 1, ts_], qtf[:, ts_], qts[h][:, 0, ts_]
                    )

            # ------- Phase B: attention + transpose + c_proj, per tq block ----
            proj_pending = []
            emit_proj_fns = {}
            for jb in (3, 2, 1, 0):
                cols = slice(jb * 512, (jb + 1) * 512)
                def emit_y(h, p_sbs, y_ps):
                    for q in range(4):
                        jt = 4 * jb + q
                        for kt in range(jt + 1):
                            nc.tensor.matmul(
                                y_ps[:, q, :],
                                lhsT=p_sbs[kt // 2][:, kt % 2, q * 128 : (q + 1) * 128],
                                rhs=v_aug[:, kt, h, :],
                                start=(kt == 0),
                                stop=(kt == jt),
                            )
                    rn = rnp.tile([128, 4], F32, tag="rn", name="rn")
                    nc.vector.reciprocal(
                        rn, y_ps[:, :, 64:65].rearrange("p a b -> p (a b)")
                    )
                    rnb = rn.rearrange("p (a b) -> p a b", b=1).broadcast_to(
                        [128, 4, 64]
                    )
                    nc.vector.tensor_mul(
                        y_t[:, 4 * jb : 4 * jb + 4, h, :], y_ps[:, :, 0:64], rnb
                    )

                pending = None
                for h in range(HPC):
                    y_ps = psy.tile([128, 4, 65], F32, tag="y1", name="y_ps")
                    p_sbs = []
                    for p in range(2 * jb + 2):
                        c0 = max(0, 256 * p - 512 * jb)
                        csl = slice(jb * 512 + c0, (jb + 1) * 512)
                        sps = ps2.tile([128, 2, 512], F32, tag="s2", name="sps")
                        for s in range(2):
                            kt = 2 * p + s
                            ksl = slice(kt * 128, (kt + 1) * 128)
                            ktb = kts[h][:, ksl].rearrange(
                                "p (j m) -> p j m", j=1
                            ).broadcast_to([128, 2, 128])
                            ktlob = ktlos[h][:, ksl].rearrange(
                                "p (j m) -> p j m", j=1
                            ).broadcast_to([128, 2, 128])
                            nc.tensor.matmul(
                                sps[:, s, c0:512],
                                lhsT=ktb,
                                rhs=qts[h][:, :, csl],
                                start=True,
                                stop=False,
                                perf_mode=PM.DoubleRow,
                            )
                            d0 = 128 * kt - 512 * jb  # diag block col offset
                            if 0 <= d0 < 512:
                                nc.tensor.matmul(
                                    sps[:, s, d0 : d0 + 128],
                                    lhsT=mlow,
                                    rhs=idsp,
                                    start=False,
                                    stop=False,
                                    perf_mode=PM.DoubleRow,
                                )
                                if d0 > c0:
                                    # fully-masked strip left of the diagonal
                                    nc.tensor.matmul(
                                        sps[:, s, c0:d0],
                                        lhsT=mful,
                                        rhs=idsp[:, :, 0 : d0 - c0],
                                        start=False,
                                        stop=False,
                                        perf_mode=PM.DoubleRow,
                                    )
                            nc.tensor.matmul(
                                sps[:, s, c0:512],
                                lhsT=ktlob,
                                rhs=qts[h][:, :, csl],
                                start=False,
                                stop=True,
                                perf_mode=PM.DoubleRow,
                            )
                        p_sb = phb.tile([128, 2, 512], BF16, tag="p", name="p_sb")
                        ex = nc.scalar.activation(
                            p_sb[:, :, c0:512],
                            sps[:, :, c0:512],
                            AF.Exp,
                            scale=c_inv8,
                            bias=c_bias,
                        )
                        if act_groups["L"]:
                            add_dep_helper(ex.ins, act_groups["L"][-1].ins, sync=False,
                                           reason="phase-B exps after phase-A Lns (ACT table)")
                            act_groups["L"] = []
                        p_sbs.append(p_sb)
                    if h == 0 and proj_pending:
                        emit_proj_fns[proj_pending.pop()]()
                    if pending is not None:
                        emit_y(*pending)
                    pending = (h, p_sbs, y_ps)
                emit_y(*pending)
                # transpose 4 t-tiles + c_proj + stage + dma out (deferred:
                # emitted after the next jb's first head S/exp so PE's
                # in-order queue doesn't stall ACT at jb boundaries)
                def emit_proj(jb=jb, last=False):
                  for q in range(4):
                    tt = 4 * jb + q
                    tsl = slice(tt * 128, (tt + 1) * 128)
                    pc = psc.tile([128, 2, 128], BF16, tag="pc", name="pc")
                    for cc in range(2):
                        nc.tensor.matmul(
                            pc[:, cc, :],
                            lhsT=y_t[:, tt, 2 * cc : 2 * cc + 2, :],
                            rhs=i128,
                            start=True,
                            stop=True,
                            is_transpose=True,
                        )
                    nc.vector.tensor_copy(out=yT[:, :, tsl], in_=pc)
                  for q in range(4):
                    tt = 4 * jb + q
                    tsl = slice(tt * 128, (tt + 1) * 128)
                    ost = ostage.tile([128, 1024], F32, tag="o", name="ost")
                    for eh in range(2):
                        po = psc.tile([128, 512], F32, tag="pc", name="po")
                        for cc in range(2):
                            nc.tensor.matmul(
                                po,
                                lhsT=yT[:, cc, tsl],
                                rhs=w2[:, cc, eh * 512 : (eh + 1) * 512],
                                start=(cc == 0),
                                stop=(cc == 1),
                            )
                        if last and (2 * q + eh) % 2 == 1:
                            nc.scalar.copy(ost[:, eh * 512 : (eh + 1) * 512], po)
                        else:
                            nc.vector.tensor_copy(
                                out=ost[:, eh * 512 : (eh + 1) * 512], in_=po
                            )
                    deng = nc.sync if q % 2 == 0 else nc.gpsimd
                    deng.dma_start(out_d[tt], ost)
                emit_proj_fns[jb] = emit_proj
                proj_pending.append(jb)
            while proj_pending:
                emit_proj_fns[proj_pending.pop()](last=True)
            rn_ctx.__exit__(None, None, None)
            ost_ctx.__exit__(None, None, None)
            phb_ctx.__exit__(None, None, None)
            abp_ctx.__exit__(None, None, None)
            a5_ctx.__exit__(None, None, None)
            mupool_ctx.__exit__(None, None, None)

    nc.compile()
    return nc


def make_inputs(x, w_attn, w_proj, delta):
    """Host-side prep: per-core input dicts (core = b*4 + g)."""
    x = np.asarray(x, dtype=np.float32)
    w_attn = np.asarray(w_attn, dtype=np.float32)
    w_proj = np.asarray(w_proj, dtype=np.float32)
    delta = np.asarray(delta, dtype=np.float32)

    inv_freq = 1.0 / (BASE ** (np.arange(D, dtype=np.float32) / D))
    t = np.arange(T, dtype=np.float64)
    freqs = (t[:, None] * inv_freq[None, :].astype(np.float64)).astype(np.float32)
    cosT = np.cos(freqs).T.astype(np.float32)  # (D, T)
    sinT = np.sin(freqs).T.astype(np.float32)
    trig = np.concatenate([cosT, sinT], axis=0).astype(ml_dtypes.bfloat16)
    d = np.clip(delta, -2.0 * math.pi, 0.0)

    # fp8 split of x per batch: [4cc, 128p, 2j, T] with c = cc*256+j*128+p
    def to_dr(mat):  # mat (T, C) -> (4, 128, 2, T)
        m = mat.T.reshape(4, 2, 128, T)  # (cc, j, p, t)
        return np.ascontiguousarray(m.transpose(0, 2, 1, 3))

    xw = [None] * B
    for b in range(B):
        xb = x[b]  # (T, C)
        x_hi = xb.astype(E4)
        x_lo = (xb - x_hi.astype(np.float32)).astype(E4)
        xw[b] = (to_dr(x_hi.astype(np.float32)).astype(E4),
                 to_dr(x_lo.astype(np.float32)).astype(E4))

    qw = w_attn[:C].reshape(H, D, C)
    kw = w_attn[C : 2 * C].reshape(H, D, C)
    vw = w_attn[2 * C :].reshape(H, D, C)

    # mask constants
    pp, jj, tk = np.meshgrid(
        np.arange(64), np.arange(2), np.arange(128), indexing="ij"
    )
    f = jj * 64 + pp
    mlow = np.where(tk > f, -240.0, 0.0).astype(E4)
    mful = np.full((64, 2, 128), -240.0, dtype=np.float32).astype(E4)
    idsp = (tk == f).astype(np.float32).astype(E4)
    i128 = np.eye(128, dtype=np.float32).astype(ml_dtypes.bfloat16)

    in_maps = []
    for core in range(N_CORES):
        b, g = divmod(core, HPC)
        heads = list(range(HPC * g, HPC * g + HPC))

        # wqk: (4cc, 128p, 2j, 512): col = h*128 + r; r<64 q_d else k_d
        wqk_full = np.empty((C, 512), dtype=np.float32)  # (c, col)
        for hi_, hg in enumerate(heads):
            wqk_full[:, hi_ * 128 : hi_ * 128 + 64] = qw[hg].T * WSCALE
            wqk_full[:, hi_ * 128 + 64 : hi_ * 128 + 128] = kw[hg].T * WSCALE
        wqk8 = wqk_full.astype(E4)
        wqk_dr = np.ascontiguousarray(
            wqk8.reshape(4, 2, 128, 512).transpose(0, 2, 1, 3)
        )

        wv_full = (
            vw[HPC * g : HPC * g + HPC].reshape(256, C).T * WSCALE
        )  # (c, 256)
        wv_hi = wv_full.astype(E4)
        wv_lo = (wv_full - wv_hi.astype(np.float32)).astype(E4)
        wvh_dr = np.ascontiguousarray(
            wv_hi.reshape(4, 2, 128, 256).transpose(0, 2, 1, 3)
        )
        wvl_dr = np.ascontiguousarray(
            wv_lo.reshape(4, 2, 128, 256).transpose(0, 2, 1, 3)
        )

        ab = np.stack(
            [
                np.concatenate(
                    [
                        np.cos(freqs + d[hg][None, :]).T,
                        np.sin(freqs + d[hg][None, :]).T,
                    ],
                    axis=0,
                ).astype(ml_dtypes.bfloat16)
                for hg in heads
            ],
            axis=0,
        )  # (4, 128, T)

        # w2: (2cc, 128p, 1024e): channel c_local = cc*128 + p of this group's
        # 256 y channels; y channel (h_local, dd) flattened h_local*64+dd
        w2g = w_proj[:, 256 * g : 256 * (g + 1)]  # (e, 256)
        w2_dr = np.ascontiguousarray((w2g.T / WSCALE).reshape(2, 128, 1024)).astype(ml_dtypes.bfloat16)

        in_maps.append(
            {
                "xhi": xw[b][0],
                "xlo": xw[b][1],
                "wqk": wqk_dr,
                "wvh": wvh_dr,
                "wvl": wvl_dr,
                "trig": trig,
                "ab": ab,
                "mlow": mlow,
                "mful": mful,
                "idsp": idsp,
                "i128": i128,
                "w2": w2_dr,
            }
        )
    return in_maps


_NC_CACHE = []


def _get_nc():
    if not _NC_CACHE:
        _NC_CACHE.append(build_module())
    return _NC_CACHE[0]


def kernel(x, w_attn, w_proj, delta, _trace=False):
    in_maps = make_inputs(x, w_attn, w_proj, delta)
    nc = _get_nc()
    res = None
    outs = None
    last_err = None
    for attempt in range(3):
        try:
            res = bass_utils.run_bass_kernel_spmd(
                nc, in_maps, core_ids=list(range(N_CORES)), trace=_trace
            )
            outs = [np.asarray(r["out"]).reshape(T, C) for r in res.results]
            break
        except Exception as e:
            last_err = e
            if "unrecoverable" not in str(e).lower() or attempt == 2:
                raise
            import time as _time

            _time.sleep(2.0)
    assert outs is not None, last_err
    if _trace:
        kernel.last_results = res
    full = np.zeros((B, T, C), dtype=np.float32)
    for core in range(N_CORES):
        full[core // HPC] += outs[core]
    return full
